# revision 9
# baseline (speedup 1.0000x reference)
"""DeepseekV3 MoE experts kernel for 8 Trainium2 NeuronCores.

Problem: every expert processes the FULL token set.
  g = x @ w_gate[e].T ; u = x @ w_up[e].T ; h = silu(g)*u
  out[e] = h @ w_down[e].T ;  concat over e -> [E*T, H]

Sharding: expert-parallel. Core c owns experts {2c, 2c+1}; hidden_states is
replicated; outputs are concatenated host-side (no on-device collectives).

Per-core compute (per expert e, with xT = x.T resident in SBUF):
  phase 1: gT[m*128:(m+1)*128, :] = wg_slab[m][:,k,:].T @ xT[:,k,:] (acc over k)
  phase 2: same for uT; hT = silu(gT) * uT  (in [I, T] layout, no transposes)
  phase 3: out[mt*128.., nslice] = hT[:,k,mtslice].T @ wd_slab[n][:,k,:]

All matmul operands are bf16 (host-cast); PSUM accumulation is fp32 and the
output is written back in fp32.

DMA plan (the mid-kernel PE stream is already at roofline; the wins are at
the edges):
  - Weights are laid out host-side in consumption-order slabs: wg/wu as
    [E, MO, 128, KO, 128] m-major slabs (0.5 MiB each; phase-1 m-chunk m
    needs only slab m), wd as [E, NH, 128, MO, 512] n-major slabs (1 MiB).
    The very first slab is split so the first matmul is gated on ~160 KiB.
  - All weight DMAs go on the Sync HWDGE ring in exact consumption order.
  - xT and all output stores go on the Scalar HWDGE ring (the second
    physical HWDGE ring), so the x ramp runs in parallel with the weight
    ramp and output stores never head-of-line block weight prefetches.
  - PSUM->SBUF output copies run on the (otherwise idle) Vector engine.
  - The last output group is split in two so the final store's HBM write
    receipt (which gates the kernel drain) covers half the bytes.

Self-contained: shapes hardcoded; inputs are the full arrays from
setup_inputs(); returns the full [4096, 2048] fp32 output.
"""

import numpy as np
import ml_dtypes

E, T, H, I = 16, 256, 2048, 1024
N_CORES = 8
E_PER = E // N_CORES  # 2
P = 128
KO = H // P  # 16 k-chunks for phases 1/2
MO = I // P  # 8 m-chunks for phases 1/2 (= k-chunks for phase 3)
TO = T // P  # 2 m-chunks for phase 3
NS = 512  # n-slice width for phase 3
NH = H // NS  # 4

WARMUP_MM = 20  # matmuls on a scratch tile bridging the preamble to the
# point where x + the first wg half-slab have landed (~12us). Sized so PE
# activity is CONTINUOUS from the first warmup MM to the first real MM: any
# gap resets the HAM activity window and the clock gate then flips to 8/8
# only ~3.4us after the last gap (measured: a 12-MM warmup left the whole
# ramp at 1.2 GHz until 21us, costing ~4.6us).

_CACHE: dict = {}


def _build_program(sim_compat=False):
    # sim_compat: CoreSim lacks the Silu LUT — express silu as
    # sigmoid(g)*g with an extra DVE multiply. HW uses the fused Silu op.
    import concourse.mybir as mybir
    import concourse.tile as tile
    from concourse import bacc

    dt = mybir.dt.bfloat16
    f32 = mybir.dt.float32
    AF = mybir.ActivationFunctionType

    nc = bacc.Bacc(None, target_bir_lowering=False, debug=False)

    xT = nc.dram_tensor("xT", [P, KO, T], dt, kind="ExternalInput")[:]
    wg = nc.dram_tensor("wg", [E_PER, MO, P, KO, P], dt, kind="ExternalInput")[:]
    wu = nc.dram_tensor("wu", [E_PER, MO, P, KO, P], dt, kind="ExternalInput")[:]
    wd = nc.dram_tensor("wd", [E_PER, NH, P, MO, NS], dt, kind="ExternalInput")[:]
    # output is stored bf16 (halves HBM write traffic; host upcasts to f32)
    out = nc.dram_tensor("out", [E_PER, TO, P, H], dt, kind="ExternalOutput")[:]

    with tile.TileContext(nc) as tc:
        with (
            tc.tile_pool(name="xp", bufs=1) as xp,
            tc.tile_pool(name="wgp", bufs=12) as wgp,
            tc.tile_pool(name="wup", bufs=11) as wup,
            tc.tile_pool(name="wdp", bufs=7) as wdp,
            tc.tile_pool(name="hp", bufs=2) as hp,
            tc.tile_pool(name="gp", bufs=8) as gp,
            tc.tile_pool(name="op", bufs=8) as outp,
            tc.tile_pool(name="ps", bufs=8, space="PSUM") as ps,
        ):
            # PE warm-up on a zeroed scratch tile: flips the HAM clock gate
            # toward 8/8 while the first input DMAs are still in flight.
            warm = xp.tile([P, T], dt, tag="warm")
            nc.vector.memset(warm[:], 0.0)
            wps = ps.tile([P, T], f32, tag="ps")
            for _ in range(WARMUP_MM):
                nc.tensor.matmul(wps[:], warm[:, :P], warm[:], start=True, stop=True)

            xtiles = []  # (k0, kq, tile)
            wslabs: dict = {}  # (which, e, m) -> list of (k0, kq, tile)
            wdslabs: dict = {}  # (e, n) -> tile

            def issue_x(k0, kq, tag, eng):
                # x rides the Scalar HWDGE ring, in parallel with the
                # weight stream on the Sync ring during the ramp
                t_ = xp.tile([P, kq, T], dt, tag=tag)
                eng.dma_start(t_[:], xT[:, k0 : k0 + kq, :])
                xtiles.append((k0, kq, t_))

            def xslice(k):
                for k0, kq, t_ in xtiles:
                    if k0 <= k < k0 + kq:
                        return t_[:, k - k0, :]
                raise KeyError(k)

            def issue_w(pool, src, which, e, m, k0, kq, eng):
                t_ = pool.tile([P, kq, P], dt, tag=pool.name)
                eng.dma_start(t_[:], src[e, m, :, k0 : k0 + kq, :])
                wslabs.setdefault((which, e, m), []).append((k0, kq, t_))

            def wslice(which, e, m, k):
                for k0, kq, t_ in wslabs[(which, e, m)]:
                    if k0 <= k < k0 + kq:
                        return t_[:, k - k0, :]
                raise KeyError((which, e, m, k))

            def issue_wd(e, n, eng):
                t_ = wdp.tile([P, MO, NS], dt, tag="wd")
                eng.dma_start(t_[:], wd[e, n, :, :, :])
                wdslabs[(e, n)] = t_

            # DMA issue order == consumption order. Weights on the Sync
            # ring; x on the Scalar ring so both ramps run in parallel.
            # The leading pieces are small so the first matmul starts as
            # soon as ~160 KiB has landed.
            # Weights ride the Sync HWDGE ring in consumption order, except
            # expert 0's odd slabs which go via the GpSimd SWDGE queue: it
            # has its OWN 8 completion-sem lanes, so this doubles in-flight
            # transfer depth during the ramp without the cross-ring sem
            # coupling that made Sync/Scalar HWDGE alternation ~20us worse
            # (the 8 HWDGE lanes are round-robined globally in trace order,
            # so HWDGE alternation blocks each sequencer on the other
            # ring's receipts). Output stores are on the Scalar ring so
            # they never head-of-line block weight prefetches.
            # (Splitting odd slabs onto the GpSimd SWDGE queue was also
            # tried: ~4us worse — SWDGE transfers land later, alternating
            # starvation through phase 1/2.)
            def wring(i, e=0):
                return nc.sync

            # Stream head: x in 4 quarter pieces interleaved with the first
            # wg half-slabs, ALL on the sync ring in consumption order. The
            # first matmul is gated on x[k<4] + wg[0,0][k<8] (~0.5 MiB), and
            # each subsequent need lands just-in-time during the ramp.
            issue_x(0, 4, "x0", nc.sync)
            issue_w(wgp, wg, "wg", 0, 0, 0, KO // 2, nc.sync)
            issue_x(4, 4, "x1", nc.sync)
            issue_w(wgp, wg, "wg", 0, 0, KO // 2, KO // 2, nc.sync)
            issue_x(8, 4, "x2", nc.sync)
            issue_x(12, 4, "x3", nc.sync)
            # all of e0's wg slabs ride as k-halves: each m-loop can start
            # on the first half while the second is still in flight, which
            # spreads the ramp's supply-trailing into sub-us stalls
            for m in range(1, MO):
                issue_w(wgp, wg, "wg", 0, m, 0, KO // 2, wring(m))
                issue_w(wgp, wg, "wg", 0, m, KO // 2, KO // 2, wring(m))
            for m in range(MO):
                issue_w(wup, wu, "wu", 0, m, 0, KO, wring(m))
            # (Moving wd to the Scalar ring was tried: ~12us worse — the
            # scalar queue drains from ~9us at packet round-robin and
            # steals half the plane exactly when wg/wu-e0 are critical.)
            for n in range(NH):
                issue_wd(0, n, nc.sync)
            for e in range(1, E_PER):
                for m in range(MO):
                    issue_w(wgp, wg, "wg", e, m, 0, KO, nc.sync)
                for m in range(MO):
                    issue_w(wup, wu, "wu", e, m, 0, KO, nc.sync)
                for n in range(NH):
                    issue_wd(e, n, nc.sync)

            for e in range(E_PER):
                hT = hp.tile([P, MO, T], dt, tag="h")
                # all gate m-chunks first: the PE is gated only by the wg
                # stream, never by wu (which lands while these run)
                gss = []
                for m in range(MO):
                    pg = ps.tile([P, T], f32, tag="ps")
                    for k in range(KO):
                        nc.tensor.matmul(
                            pg[:],
                            wslice("wg", e, m, k),
                            xslice(k),
                            start=(k == 0),
                            stop=(k == KO - 1),
                        )
                    gs = gp.tile([P, T], f32, tag="g")
                    if sim_compat:
                        nc.scalar.activation(gs[:], pg[:], AF.Sigmoid)
                        nc.vector.tensor_tensor(
                            gs[:], gs[:], pg[:], mybir.AluOpType.mult
                        )
                    else:
                        nc.scalar.activation(gs[:], pg[:], AF.Silu)
                    gss.append(gs)
                for m in range(MO):
                    pu = ps.tile([P, T], f32, tag="ps")
                    for k in range(KO):
                        nc.tensor.matmul(
                            pu[:],
                            wslice("wu", e, m, k),
                            xslice(k),
                            start=(k == 0),
                            stop=(k == KO - 1),
                        )
                    nc.vector.tensor_tensor(
                        hT[:, m, :], gss[m][:], pu[:], mybir.AluOpType.mult
                    )
                for n in range(NH):
                    wdt = wdslabs[(e, n)]
                    for mt in range(TO):
                        last = e == E_PER - 1 and n == NH - 1 and mt == TO - 1
                        # the final group is stored as two half-width
                        # slices so the last HBM write receipt is smaller
                        nsub = 2 if last else 1
                        sub = NS // nsub
                        for s in range(nsub):
                            po = ps.tile([P, sub], f32, tag="ps")
                            for k in range(MO):
                                nc.tensor.matmul(
                                    po[:],
                                    hT[:, k, mt * P : (mt + 1) * P],
                                    wdt[:, k, s * sub : (s + 1) * sub],
                                    start=(k == 0),
                                    stop=(k == MO - 1),
                                )
                            # PSUM->SBUF copy casts f32 -> bf16 on the DVE
                            ot = outp.tile([P, sub], dt, tag="o")
                            nc.vector.tensor_copy(ot[:], po[:])
                            # the very last store rides the (by then idle)
                            # Sync ring so the two final store dispatches
                            # run on different sequencers in parallel —
                            # trims the end-of-kernel write-receipt tail
                            oeng = nc.sync if (last and s == nsub - 1) else nc.scalar
                            oeng.dma_start(
                                out[e, mt, :, n * NS + s * sub : n * NS + (s + 1) * sub],
                                ot[:],
                            )

    nc.compile()
    return nc


def get_program(sim_compat=False):
    key = ("nc", sim_compat)
    if key not in _CACHE:
        _CACHE[key] = _build_program(sim_compat=sim_compat)
    return _CACHE[key]


def _prep_in_maps(hidden_states, w_gate, w_up, w_down):
    bf16 = ml_dtypes.bfloat16
    x = np.asarray(hidden_states, dtype=np.float32)
    wg = np.asarray(w_gate, dtype=np.float32)
    wu = np.asarray(w_up, dtype=np.float32)
    wd = np.asarray(w_down, dtype=np.float32)

    # xT: [H, T] -> [128, KO, T], partition p + chunk k <-> H index k*128+p
    xt = np.ascontiguousarray(
        x.T.reshape(KO, P, T).transpose(1, 0, 2).astype(bf16)
    )
    # w_gate/w_up: [E, I, H] -> m-major slabs [E, MO, P, KO, 128]:
    #   slab[e, m, p, k, c] = W.T[k*128+p, m*128+c] = w[e, m*128+c, k*128+p]
    wgt = np.ascontiguousarray(
        wg.reshape(E, MO, P, KO, P).transpose(0, 1, 4, 3, 2).astype(bf16)
    )
    wut = np.ascontiguousarray(
        wu.reshape(E, MO, P, KO, P).transpose(0, 1, 4, 3, 2).astype(bf16)
    )
    # w_down: [E, H, I] -> n-major slabs [E, NH, P, MO, NS]:
    #   slab[e, n, p, k, c] = W.T[k*128+p, n*512+c] = w[e, n*512+c, k*128+p]
    wdt = np.ascontiguousarray(
        wd.reshape(E, NH, NS, MO, P).transpose(0, 1, 4, 3, 2).astype(bf16)
    )

    in_maps = []
    for c in range(N_CORES):
        sl = slice(c * E_PER, (c + 1) * E_PER)
        in_maps.append(
            {
                "xT": xt,
                "wg": np.ascontiguousarray(wgt[sl]),
                "wu": np.ascontiguousarray(wut[sl]),
                "wd": np.ascontiguousarray(wdt[sl]),
            }
        )
    return in_maps


def kernel(hidden_states, w_gate, w_up, w_down, _trace=False, _trace_kwargs=None):
    from concourse.bass_utils import run_bass_kernel_spmd

    nc = get_program()
    in_maps = _prep_in_maps(hidden_states, w_gate, w_up, w_down)
    kwargs = {}
    if _trace:
        kwargs = dict(trace=True, **(_trace_kwargs or {}))
    res = run_bass_kernel_spmd(nc, in_maps, core_ids=list(range(N_CORES)), **kwargs)
    out = np.concatenate(
        [
            res.results[c]["out"].astype(np.float32).reshape(E_PER * T, H)
            for c in range(N_CORES)
        ],
        axis=0,
    )
    if _trace:
        _CACHE["last_results"] = res
    return out



# revision 15
# speedup vs baseline: 1.0049x; 1.0049x over previous
"""DeepseekV3 MoE experts kernel for 8 Trainium2 NeuronCores.

Problem: every expert processes the FULL token set.
  g = x @ w_gate[e].T ; u = x @ w_up[e].T ; h = silu(g)*u
  out[e] = h @ w_down[e].T ;  concat over e -> [E*T, H]

Sharding: expert-parallel. Core c owns experts {2c, 2c+1}; hidden_states is
replicated; outputs are concatenated host-side (no on-device collectives).

Per-core compute (per expert e, with xT = x.T resident in SBUF):
  phase 1: gT[m*128:(m+1)*128, :] = wg_slab[m][:,k,:].T @ xT[:,k,:] (acc over k)
  phase 2: same for uT; hT = silu(gT) * uT  (in [I, T] layout, no transposes)
  phase 3: out[mt*128.., nslice] = hT[:,k,mtslice].T @ wd_slab[n][:,k,:]

All matmul operands are bf16 (host-cast); PSUM accumulation is fp32 and the
output is written back in fp32.

DMA plan (the mid-kernel PE stream is already at roofline; the wins are at
the edges):
  - Weights are laid out host-side in consumption-order slabs: wg/wu as
    [E, MO, 128, KO, 128] m-major slabs (0.5 MiB each; phase-1 m-chunk m
    needs only slab m), wd as [E, NH, 128, MO, 512] n-major slabs (1 MiB).
    The very first slab is split so the first matmul is gated on ~160 KiB.
  - All weight DMAs go on the Sync HWDGE ring in exact consumption order.
  - xT and all output stores go on the Scalar HWDGE ring (the second
    physical HWDGE ring), so the x ramp runs in parallel with the weight
    ramp and output stores never head-of-line block weight prefetches.
  - PSUM->SBUF output copies run on the (otherwise idle) Vector engine.
  - The last output group is split in two so the final store's HBM write
    receipt (which gates the kernel drain) covers half the bytes.

Self-contained: shapes hardcoded; inputs are the full arrays from
setup_inputs(); returns the full [4096, 2048] fp32 output.
"""

import numpy as np
import ml_dtypes

E, T, H, I = 16, 256, 2048, 1024
N_CORES = 8
E_PER = E // N_CORES  # 2
P = 128
KO = H // P  # 16 k-chunks for phases 1/2
MO = I // P  # 8 m-chunks for phases 1/2 (= k-chunks for phase 3)
TO = T // P  # 2 m-chunks for phase 3
NS = 512  # n-slice width for phase 3
NH = H // NS  # 4

WARMUP_MM = 26  # matmuls on a scratch tile bridging the preamble to the
# point where x + the first wg half-slab have landed (~12us). Sized so PE
# activity is CONTINUOUS from the first warmup MM to the first real MM: any
# gap resets the HAM activity window and the clock gate then flips to 8/8
# only ~3.4us after the last gap (measured: a 12-MM warmup left the whole
# ramp at 1.2 GHz until 21us, costing ~4.6us).

_CACHE: dict = {}


def _build_program(sim_compat=False):
    # sim_compat: CoreSim lacks the Silu LUT — express silu as
    # sigmoid(g)*g with an extra DVE multiply. HW uses the fused Silu op.
    import concourse.mybir as mybir
    import concourse.tile as tile
    from concourse import bacc

    dt = mybir.dt.bfloat16
    f32 = mybir.dt.float32
    AF = mybir.ActivationFunctionType

    nc = bacc.Bacc(None, target_bir_lowering=False, debug=False)

    xT = nc.dram_tensor("xT", [P, KO, T], dt, kind="ExternalInput")[:]
    wg = nc.dram_tensor("wg", [E_PER, MO, P, KO, P], dt, kind="ExternalInput")[:]
    wu = nc.dram_tensor("wu", [E_PER, MO, P, KO, P], dt, kind="ExternalInput")[:]
    wd = nc.dram_tensor("wd", [E_PER, NH, P, MO, NS], dt, kind="ExternalInput")[:]
    # output is stored bf16 (halves HBM write traffic; host upcasts to f32)
    out = nc.dram_tensor("out", [E_PER, TO, P, H], dt, kind="ExternalOutput")[:]

    with tile.TileContext(nc) as tc:
        with (
            tc.tile_pool(name="xp", bufs=1) as xp,
            tc.tile_pool(name="wgp", bufs=12) as wgp,
            tc.tile_pool(name="wgp2", bufs=4) as wgp2,
            tc.tile_pool(name="wup", bufs=4) as wup,
            tc.tile_pool(name="wdp", bufs=6) as wdp,
            tc.tile_pool(name="hp", bufs=2) as hp,
            tc.tile_pool(name="gp", bufs=8) as gp,
            tc.tile_pool(name="op", bufs=8) as outp,
            tc.tile_pool(name="ps", bufs=8, space="PSUM") as ps,
        ):
            # PE warm-up on a zeroed scratch tile: flips the HAM clock gate
            # toward 8/8 while the first input DMAs are still in flight.
            warm = xp.tile([P, T], dt, tag="warm")
            nc.vector.memset(warm[:], 0.0)
            wps = ps.tile([P, T], f32, tag="ps")
            for _ in range(WARMUP_MM):
                nc.tensor.matmul(wps[:], warm[:, :P], warm[:], start=True, stop=True)

            xtiles = []  # (k0, kq, tile)
            wslabs: dict = {}  # (which, e) -> list of (m0, mq, k0, kq, tile)
            wdslabs: dict = {}  # (e, n) -> tile

            def issue_x(k0, kq, tag, eng):
                t_ = xp.tile([P, kq, T], dt, tag=tag)
                eng.dma_start(t_[:], xT[:, k0 : k0 + kq, :])
                xtiles.append((k0, kq, t_))

            def xslice(k):
                for k0, kq, t_ in xtiles:
                    if k0 <= k < k0 + kq:
                        return t_[:, k - k0, :]
                raise KeyError(k)

            def issue_w(pool, src, which, e, m, k0, kq, eng):
                # single-m slab piece [P, kq, P]
                t_ = pool.tile([P, kq, P], dt, tag=pool.name)
                eng.dma_start(t_[:], src[e, m, :, k0 : k0 + kq, :])
                wslabs.setdefault((which, e), []).append((m, 1, k0, kq, t_))

            def issue_w2(pool, src, which, e, m0, eng):
                # double-m slab [P, 2, KO, P] (1 MiB): one dispatch covers
                # two m-chunks, so the DMA stream is never paced by the
                # ~0.6us/dispatch NX cost of DMA_DIRECT2D instructions
                t_ = pool.tile([P, 2, KO, P], dt, tag=pool.name)
                eng.dma_start(
                    t_[:], src[e, m0 : m0 + 2, :, :, :].transpose([1, 0, 2, 3])
                )
                wslabs.setdefault((which, e), []).append((m0, 2, 0, KO, t_))

            def wslice(which, e, m, k):
                for m0, mq, k0, kq, t_ in wslabs[(which, e)]:
                    if m0 <= m < m0 + mq and k0 <= k < k0 + kq:
                        if mq == 1:
                            return t_[:, k - k0, :]
                        return t_[:, m - m0, k - k0, :]
                raise KeyError((which, e, m, k))

            def issue_wd(e, n, eng):
                t_ = wdp.tile([P, MO, NS], dt, tag="wd")
                eng.dma_start(t_[:], wd[e, n, :, :, :])
                wdslabs[(e, n)] = t_

            # DMA issue order == consumption order. Weights on the Sync
            # ring; x on the Scalar ring so both ramps run in parallel.
            # The leading pieces are small so the first matmul starts as
            # soon as ~160 KiB has landed.
            # Weights ride the Sync HWDGE ring in consumption order, except
            # expert 0's odd slabs which go via the GpSimd SWDGE queue: it
            # has its OWN 8 completion-sem lanes, so this doubles in-flight
            # transfer depth during the ramp without the cross-ring sem
            # coupling that made Sync/Scalar HWDGE alternation ~20us worse
            # (the 8 HWDGE lanes are round-robined globally in trace order,
            # so HWDGE alternation blocks each sequencer on the other
            # ring's receipts). Output stores are on the Scalar ring so
            # they never head-of-line block weight prefetches.
            # (Splitting odd slabs onto the GpSimd SWDGE queue was also
            # tried: ~4us worse — SWDGE transfers land later, alternating
            # starvation through phase 1/2.)
            def wring(i, e=0):
                return nc.sync

            # Stream head: x in 4 quarter pieces interleaved with the first
            # wg half-slabs, ALL on the sync ring in consumption order. The
            # first matmul is gated on x[k<4] + wg[0,0][k<8] (~0.5 MiB), and
            # each subsequent need lands just-in-time during the ramp.
            issue_x(0, 4, "x0", nc.sync)
            issue_w(wgp, wg, "wg", 0, 0, 0, KO // 2, nc.sync)
            issue_x(4, 4, "x1", nc.sync)
            issue_w(wgp, wg, "wg", 0, 0, KO // 2, KO // 2, nc.sync)
            issue_x(8, 4, "x2", nc.sync)
            issue_x(12, 4, "x3", nc.sync)
            # early slabs split into k-halves: each m-loop can start on the
            # first half while the second is still in flight (earlier sems
            # during the DMA ramp)
            for m in range(1, 4):
                issue_w(wgp, wg, "wg", 0, m, 0, KO // 2, wring(m))
                issue_w(wgp, wg, "wg", 0, m, KO // 2, KO // 2, wring(m))
            for m in range(4, MO):
                issue_w(wgp, wg, "wg", 0, m, 0, KO, wring(m))
            # once past the ramp, everything rides as 1 MiB double-slabs
            for m0 in range(0, MO, 2):
                issue_w2(wup, wu, "wu", 0, m0, nc.sync)
            # (Moving wd to the Scalar ring was tried: ~12us worse — the
            # scalar queue drains from ~9us at packet round-robin and
            # steals half the plane exactly when wg/wu-e0 are critical.)
            for n in range(NH):
                issue_wd(0, n, nc.sync)
            for e in range(1, E_PER):
                for m0 in range(0, MO, 2):
                    issue_w2(wgp2, wg, "wg", e, m0, nc.sync)
                for m0 in range(0, MO, 2):
                    issue_w2(wup, wu, "wu", e, m0, nc.sync)
                for n in range(NH):
                    issue_wd(e, n, nc.sync)

            for e in range(E_PER):
                hT = hp.tile([P, MO, T], dt, tag="h")
                # all gate m-chunks first: the PE is gated only by the wg
                # stream, never by wu (which lands while these run)
                gss = []
                for m in range(MO):
                    pg = ps.tile([P, T], f32, tag="ps")
                    for k in range(KO):
                        nc.tensor.matmul(
                            pg[:],
                            wslice("wg", e, m, k),
                            xslice(k),
                            start=(k == 0),
                            stop=(k == KO - 1),
                        )
                    gs = gp.tile([P, T], f32, tag="g")
                    if sim_compat:
                        nc.scalar.activation(gs[:], pg[:], AF.Sigmoid)
                        nc.vector.tensor_tensor(
                            gs[:], gs[:], pg[:], mybir.AluOpType.mult
                        )
                    else:
                        nc.scalar.activation(gs[:], pg[:], AF.Silu)
                    gss.append(gs)
                for m in range(MO):
                    pu = ps.tile([P, T], f32, tag="ps")
                    for k in range(KO):
                        nc.tensor.matmul(
                            pu[:],
                            wslice("wu", e, m, k),
                            xslice(k),
                            start=(k == 0),
                            stop=(k == KO - 1),
                        )
                    nc.vector.tensor_tensor(
                        hT[:, m, :], gss[m][:], pu[:], mybir.AluOpType.mult
                    )
                for n in range(NH):
                    wdt = wdslabs[(e, n)]
                    for mt in range(TO):
                        last = e == E_PER - 1 and n == NH - 1 and mt == TO - 1
                        # the final group is stored as two half-width
                        # slices so the last HBM write receipt is smaller
                        nsub = 2 if last else 1
                        sub = NS // nsub
                        for s in range(nsub):
                            po = ps.tile([P, sub], f32, tag="ps")
                            for k in range(MO):
                                nc.tensor.matmul(
                                    po[:],
                                    hT[:, k, mt * P : (mt + 1) * P],
                                    wdt[:, k, s * sub : (s + 1) * sub],
                                    start=(k == 0),
                                    stop=(k == MO - 1),
                                )
                            # PSUM->SBUF copy casts f32 -> bf16 on the DVE
                            ot = outp.tile([P, sub], dt, tag="o")
                            nc.vector.tensor_copy(ot[:], po[:])
                            # the very last store rides the (by then idle)
                            # Sync ring so the two final store dispatches
                            # run on different sequencers in parallel —
                            # trims the end-of-kernel write-receipt tail
                            oeng = nc.sync if (last and s == nsub - 1) else nc.scalar
                            oeng.dma_start(
                                out[e, mt, :, n * NS + s * sub : n * NS + (s + 1) * sub],
                                ot[:],
                            )

    nc.compile()
    return nc


def get_program(sim_compat=False):
    key = ("nc", sim_compat)
    if key not in _CACHE:
        _CACHE[key] = _build_program(sim_compat=sim_compat)
    return _CACHE[key]


def _prep_in_maps(hidden_states, w_gate, w_up, w_down):
    bf16 = ml_dtypes.bfloat16
    x = np.asarray(hidden_states, dtype=np.float32)
    wg = np.asarray(w_gate, dtype=np.float32)
    wu = np.asarray(w_up, dtype=np.float32)
    wd = np.asarray(w_down, dtype=np.float32)

    # xT: [H, T] -> [128, KO, T], partition p + chunk k <-> H index k*128+p
    xt = np.ascontiguousarray(
        x.T.reshape(KO, P, T).transpose(1, 0, 2).astype(bf16)
    )
    # w_gate/w_up: [E, I, H] -> m-major slabs [E, MO, P, KO, 128]:
    #   slab[e, m, p, k, c] = W.T[k*128+p, m*128+c] = w[e, m*128+c, k*128+p]
    wgt = np.ascontiguousarray(
        wg.reshape(E, MO, P, KO, P).transpose(0, 1, 4, 3, 2).astype(bf16)
    )
    wut = np.ascontiguousarray(
        wu.reshape(E, MO, P, KO, P).transpose(0, 1, 4, 3, 2).astype(bf16)
    )
    # w_down: [E, H, I] -> n-major slabs [E, NH, P, MO, NS]:
    #   slab[e, n, p, k, c] = W.T[k*128+p, n*512+c] = w[e, n*512+c, k*128+p]
    wdt = np.ascontiguousarray(
        wd.reshape(E, NH, NS, MO, P).transpose(0, 1, 4, 3, 2).astype(bf16)
    )

    in_maps = []
    for c in range(N_CORES):
        sl = slice(c * E_PER, (c + 1) * E_PER)
        in_maps.append(
            {
                "xT": xt,
                "wg": np.ascontiguousarray(wgt[sl]),
                "wu": np.ascontiguousarray(wut[sl]),
                "wd": np.ascontiguousarray(wdt[sl]),
            }
        )
    return in_maps


def kernel(hidden_states, w_gate, w_up, w_down, _trace=False, _trace_kwargs=None):
    from concourse.bass_utils import run_bass_kernel_spmd

    nc = get_program()
    in_maps = _prep_in_maps(hidden_states, w_gate, w_up, w_down)
    kwargs = {}
    if _trace:
        kwargs = dict(trace=True, **(_trace_kwargs or {}))
    res = run_bass_kernel_spmd(nc, in_maps, core_ids=list(range(N_CORES)), **kwargs)
    out = np.concatenate(
        [
            res.results[c]["out"].astype(np.float32).reshape(E_PER * T, H)
            for c in range(N_CORES)
        ],
        axis=0,
    )
    if _trace:
        _CACHE["last_results"] = res
    return out



# revision 19
# speedup vs baseline: 1.0082x; 1.0034x over previous
"""DeepseekV3 MoE experts kernel for 8 Trainium2 NeuronCores.

Problem: every expert processes the FULL token set.
  g = x @ w_gate[e].T ; u = x @ w_up[e].T ; h = silu(g)*u
  out[e] = h @ w_down[e].T ;  concat over e -> [E*T, H]

Sharding: expert-parallel. Core c owns experts {2c, 2c+1}; hidden_states is
replicated; outputs are concatenated host-side (no on-device collectives).

Per-core compute (per expert e, with xT = x.T resident in SBUF):
  phase 1: gT[m*128:(m+1)*128, :] = wg_slab[m][:,k,:].T @ xT[:,k,:] (acc over k)
  phase 2: same for uT; hT = silu(gT) * uT  (in [I, T] layout, no transposes)
  phase 3: out[mt*128.., nslice] = hT[:,k,mtslice].T @ wd_slab[n][:,k,:]

All matmul operands are bf16 (host-cast); PSUM accumulation is fp32 and the
output is written back in fp32.

DMA plan (the mid-kernel PE stream is already at roofline; the wins are at
the edges):
  - Weights are laid out host-side in consumption-order slabs: wg/wu as
    [E, MO, 128, KO, 128] m-major slabs (0.5 MiB each; phase-1 m-chunk m
    needs only slab m), wd as [E, NH, 128, MO, 512] n-major slabs (1 MiB).
    The very first slab is split so the first matmul is gated on ~160 KiB.
  - All weight DMAs go on the Sync HWDGE ring in exact consumption order.
  - xT and all output stores go on the Scalar HWDGE ring (the second
    physical HWDGE ring), so the x ramp runs in parallel with the weight
    ramp and output stores never head-of-line block weight prefetches.
  - PSUM->SBUF output copies run on the (otherwise idle) Vector engine.
  - The last output group is split in two so the final store's HBM write
    receipt (which gates the kernel drain) covers half the bytes.

Self-contained: shapes hardcoded; inputs are the full arrays from
setup_inputs(); returns the full [4096, 2048] fp32 output.
"""

import numpy as np
import ml_dtypes

E, T, H, I = 16, 256, 2048, 1024
N_CORES = 8
E_PER = E // N_CORES  # 2
P = 128
KO = H // P  # 16 k-chunks for phases 1/2
MO = I // P  # 8 m-chunks for phases 1/2 (= k-chunks for phase 3)
TO = T // P  # 2 m-chunks for phase 3
NS = 512  # n-slice width for phase 3
NH = H // NS  # 4

WARMUP_MM = 16  # matmuls on a scratch tile bridging the preamble to the
# point where x[k<4] + the first wg half-slab have landed (~10us). Sized so
# PE activity is CONTINUOUS from the first warmup MM to the first real MM:
# any gap resets the HAM activity window and the clock gate then flips to
# 8/8 only ~3.4us after the last gap (measured: a 12-MM warmup with a
# ramp gap left the whole ramp at 1.2 GHz until 21us, costing ~4.6us).
# 16 cold MMs = ~3.4us = exactly one HAM SHORT window.

_CACHE: dict = {}


def _build_program(sim_compat=False):
    # sim_compat: CoreSim lacks the Silu LUT — express silu as
    # sigmoid(g)*g with an extra DVE multiply. HW uses the fused Silu op.
    import concourse.mybir as mybir
    import concourse.tile as tile
    from concourse import bacc

    dt = mybir.dt.bfloat16
    f32 = mybir.dt.float32
    AF = mybir.ActivationFunctionType

    nc = bacc.Bacc(None, target_bir_lowering=False, debug=False)

    xT = nc.dram_tensor("xT", [P, KO, T], dt, kind="ExternalInput")[:]
    wg = nc.dram_tensor("wg", [E_PER, MO, P, KO, P], dt, kind="ExternalInput")[:]
    wu = nc.dram_tensor("wu", [E_PER, MO, P, KO, P], dt, kind="ExternalInput")[:]
    wd = nc.dram_tensor("wd", [E_PER, NH, P, MO, NS], dt, kind="ExternalInput")[:]
    # output is stored bf16 (halves HBM write traffic; host upcasts to f32)
    out = nc.dram_tensor("out", [E_PER, TO, P, H], dt, kind="ExternalOutput")[:]

    with tile.TileContext(nc) as tc:
        with (
            tc.tile_pool(name="xp", bufs=1) as xp,
            tc.tile_pool(name="wgp", bufs=12) as wgp,
            tc.tile_pool(name="wgp2", bufs=4) as wgp2,
            tc.tile_pool(name="wup", bufs=4) as wup,
            tc.tile_pool(name="wdp", bufs=6) as wdp,
            tc.tile_pool(name="hp", bufs=2) as hp,
            tc.tile_pool(name="gp", bufs=8) as gp,
            tc.tile_pool(name="op", bufs=8) as outp,
            tc.tile_pool(name="ps", bufs=8, space="PSUM") as ps,
        ):
            # PE warm-up on a zeroed scratch tile: flips the HAM clock gate
            # toward 8/8 while the first input DMAs are still in flight.
            warm = xp.tile([P, T], dt, tag="warm")
            nc.vector.memset(warm[:], 0.0)
            wps = ps.tile([P, T], f32, tag="ps")
            for _ in range(WARMUP_MM):
                nc.tensor.matmul(wps[:], warm[:, :P], warm[:], start=True, stop=True)

            xtiles = []  # (k0, kq, tile)
            wslabs: dict = {}  # (which, e) -> list of (m0, mq, k0, kq, tile)
            wdslabs: dict = {}  # (e, n) -> tile

            def issue_x(k0, kq, tag, eng):
                t_ = xp.tile([P, kq, T], dt, tag=tag)
                eng.dma_start(t_[:], xT[:, k0 : k0 + kq, :])
                xtiles.append((k0, kq, t_))

            def xslice(k):
                for k0, kq, t_ in xtiles:
                    if k0 <= k < k0 + kq:
                        return t_[:, k - k0, :]
                raise KeyError(k)

            def issue_w(pool, src, which, e, m, k0, kq, eng):
                # single-m slab piece [P, kq, P]
                t_ = pool.tile([P, kq, P], dt, tag=pool.name)
                eng.dma_start(t_[:], src[e, m, :, k0 : k0 + kq, :])
                wslabs.setdefault((which, e), []).append((m, 1, k0, kq, t_))

            def issue_w2(pool, src, which, e, m0, eng):
                # double-m slab [P, 2, KO, P] (1 MiB): one dispatch covers
                # two m-chunks, so the DMA stream is never paced by the
                # ~0.6us/dispatch NX cost of DMA_DIRECT2D instructions
                t_ = pool.tile([P, 2, KO, P], dt, tag=pool.name)
                eng.dma_start(
                    t_[:], src[e, m0 : m0 + 2, :, :, :].transpose([1, 0, 2, 3])
                )
                wslabs.setdefault((which, e), []).append((m0, 2, 0, KO, t_))

            def wslice(which, e, m, k):
                for m0, mq, k0, kq, t_ in wslabs[(which, e)]:
                    if m0 <= m < m0 + mq and k0 <= k < k0 + kq:
                        if mq == 1:
                            return t_[:, k - k0, :]
                        return t_[:, m - m0, k - k0, :]
                raise KeyError((which, e, m, k))

            def issue_wd(e, n, eng):
                t_ = wdp.tile([P, MO, NS], dt, tag="wd")
                eng.dma_start(t_[:], wd[e, n, :, :, :])
                wdslabs[(e, n)] = t_

            # DMA issue order == consumption order. Weights on the Sync
            # ring; x on the Scalar ring so both ramps run in parallel.
            # The leading pieces are small so the first matmul starts as
            # soon as ~160 KiB has landed.
            # Weights ride the Sync HWDGE ring in consumption order, except
            # expert 0's odd slabs which go via the GpSimd SWDGE queue: it
            # has its OWN 8 completion-sem lanes, so this doubles in-flight
            # transfer depth during the ramp without the cross-ring sem
            # coupling that made Sync/Scalar HWDGE alternation ~20us worse
            # (the 8 HWDGE lanes are round-robined globally in trace order,
            # so HWDGE alternation blocks each sequencer on the other
            # ring's receipts). Output stores are on the Scalar ring so
            # they never head-of-line block weight prefetches.
            # (Splitting odd slabs onto the GpSimd SWDGE queue was also
            # tried: ~4us worse — SWDGE transfers land later, alternating
            # starvation through phase 1/2.)
            def wring(i, e=0):
                return nc.sync

            # Stream head: x in 4 quarter pieces interleaved with the first
            # wg half-slabs, ALL on the sync ring in consumption order. The
            # first matmul is gated on x[k<4] + wg[0,0][k<8] (~0.5 MiB), and
            # each subsequent need lands just-in-time during the ramp.
            issue_x(0, 4, "x0", nc.sync)
            issue_w(wgp, wg, "wg", 0, 0, 0, KO // 2, nc.sync)
            issue_x(4, 4, "x1", nc.sync)
            for m in range(1, 4):
                issue_w(wgp, wg, "wg", 0, m, 0, KO // 2, wring(m))
            issue_x(8, 4, "x2", nc.sync)
            issue_w(wgp, wg, "wg", 0, 0, KO // 2, KO // 2, nc.sync)
            issue_x(12, 4, "x3", nc.sync)
            for m in range(1, 4):
                issue_w(wgp, wg, "wg", 0, m, KO // 2, KO // 2, wring(m))
            for m in range(4, MO):
                issue_w(wgp, wg, "wg", 0, m, 0, KO, wring(m))
            # once past the ramp, everything rides as 1 MiB double-slabs
            for m0 in range(0, MO, 2):
                issue_w2(wup, wu, "wu", 0, m0, nc.sync)
            # (Moving wd to the Scalar ring was tried: ~12us worse — the
            # scalar queue drains from ~9us at packet round-robin and
            # steals half the plane exactly when wg/wu-e0 are critical.)
            for n in range(NH):
                issue_wd(0, n, nc.sync)
            for e in range(1, E_PER):
                for m0 in range(0, MO, 2):
                    issue_w2(wgp2, wg, "wg", e, m0, nc.sync)
                for m0 in range(0, MO, 2):
                    issue_w2(wup, wu, "wu", e, m0, nc.sync)
                for n in range(NH):
                    issue_wd(e, n, nc.sync)

            def silu_into(gs, pg):
                if sim_compat:
                    nc.scalar.activation(gs[:], pg[:], AF.Sigmoid)
                    nc.vector.tensor_tensor(gs[:], gs[:], pg[:], mybir.AluOpType.mult)
                else:
                    nc.scalar.activation(gs[:], pg[:], AF.Silu)

            for e in range(E_PER):
                hT = hp.tile([P, MO, T], dt, tag="h")
                # all gate m-chunks first: the PE is gated only by the wg
                # stream, never by wu (which lands while these run)
                gss = []
                if e == 0:
                    # Ramp schedule: m0-m3 run as two half-chains (k0-7,
                    # then k8-15) with their PSUM accumulators left open
                    # across the interleave. The PE consumes the stream in
                    # exact arrival order (x0, wg00a, x1, wg01a..wg03a, x2,
                    # wg00b, x3, wg01b..wg03b), so the first matmul is
                    # gated on ~0.4 MiB and every later need lands
                    # just-in-time — no multi-us cliff waiting for all of
                    # x + wg[0] before m0 can finish. silu staggering is
                    # preserved: chain m closes in the second pass.
                    pgs = [
                        ps.tile([P, T], f32, tag="ps", name=f"pg{m}")
                        for m in range(4)
                    ]
                    for m in range(4):
                        for k in range(KO // 2):
                            nc.tensor.matmul(
                                pgs[m][:],
                                wslice("wg", e, m, k),
                                xslice(k),
                                start=(k == 0),
                                stop=False,
                            )
                    for m in range(4):
                        for k in range(KO // 2, KO):
                            nc.tensor.matmul(
                                pgs[m][:],
                                wslice("wg", e, m, k),
                                xslice(k),
                                start=False,
                                stop=(k == KO - 1),
                            )
                        gs = gp.tile([P, T], f32, tag="g")
                        silu_into(gs, pgs[m])
                        gss.append(gs)
                    rest = range(4, MO)
                else:
                    rest = range(MO)
                for m in rest:
                    pg = ps.tile([P, T], f32, tag="ps")
                    for k in range(KO):
                        nc.tensor.matmul(
                            pg[:],
                            wslice("wg", e, m, k),
                            xslice(k),
                            start=(k == 0),
                            stop=(k == KO - 1),
                        )
                    gs = gp.tile([P, T], f32, tag="g")
                    silu_into(gs, pg)
                    gss.append(gs)
                for m in range(MO):
                    pu = ps.tile([P, T], f32, tag="ps")
                    for k in range(KO):
                        nc.tensor.matmul(
                            pu[:],
                            wslice("wu", e, m, k),
                            xslice(k),
                            start=(k == 0),
                            stop=(k == KO - 1),
                        )
                    nc.vector.tensor_tensor(
                        hT[:, m, :], gss[m][:], pu[:], mybir.AluOpType.mult
                    )
                for n in range(NH):
                    wdt = wdslabs[(e, n)]
                    for mt in range(TO):
                        last = e == E_PER - 1 and n == NH - 1 and mt == TO - 1
                        # the final group is stored as two half-width
                        # slices so the last HBM write receipt is smaller
                        nsub = 2 if last else 1
                        sub = NS // nsub
                        for s in range(nsub):
                            po = ps.tile([P, sub], f32, tag="ps")
                            for k in range(MO):
                                nc.tensor.matmul(
                                    po[:],
                                    hT[:, k, mt * P : (mt + 1) * P],
                                    wdt[:, k, s * sub : (s + 1) * sub],
                                    start=(k == 0),
                                    stop=(k == MO - 1),
                                )
                            # PSUM->SBUF copy casts f32 -> bf16 on the DVE
                            ot = outp.tile([P, sub], dt, tag="o")
                            nc.vector.tensor_copy(ot[:], po[:])
                            # the very last store rides the (by then idle)
                            # Sync ring so the two final store dispatches
                            # run on different sequencers in parallel —
                            # trims the end-of-kernel write-receipt tail
                            oeng = nc.sync if (last and s == nsub - 1) else nc.scalar
                            oeng.dma_start(
                                out[e, mt, :, n * NS + s * sub : n * NS + (s + 1) * sub],
                                ot[:],
                            )

    nc.compile()
    return nc


def get_program(sim_compat=False):
    key = ("nc", sim_compat)
    if key not in _CACHE:
        _CACHE[key] = _build_program(sim_compat=sim_compat)
    return _CACHE[key]


def _prep_in_maps(hidden_states, w_gate, w_up, w_down):
    bf16 = ml_dtypes.bfloat16
    x = np.asarray(hidden_states, dtype=np.float32)
    wg = np.asarray(w_gate, dtype=np.float32)
    wu = np.asarray(w_up, dtype=np.float32)
    wd = np.asarray(w_down, dtype=np.float32)

    # xT: [H, T] -> [128, KO, T], partition p + chunk k <-> H index k*128+p
    xt = np.ascontiguousarray(
        x.T.reshape(KO, P, T).transpose(1, 0, 2).astype(bf16)
    )
    # w_gate/w_up: [E, I, H] -> m-major slabs [E, MO, P, KO, 128]:
    #   slab[e, m, p, k, c] = W.T[k*128+p, m*128+c] = w[e, m*128+c, k*128+p]
    wgt = np.ascontiguousarray(
        wg.reshape(E, MO, P, KO, P).transpose(0, 1, 4, 3, 2).astype(bf16)
    )
    wut = np.ascontiguousarray(
        wu.reshape(E, MO, P, KO, P).transpose(0, 1, 4, 3, 2).astype(bf16)
    )
    # w_down: [E, H, I] -> n-major slabs [E, NH, P, MO, NS]:
    #   slab[e, n, p, k, c] = W.T[k*128+p, n*512+c] = w[e, n*512+c, k*128+p]
    wdt = np.ascontiguousarray(
        wd.reshape(E, NH, NS, MO, P).transpose(0, 1, 4, 3, 2).astype(bf16)
    )

    in_maps = []
    for c in range(N_CORES):
        sl = slice(c * E_PER, (c + 1) * E_PER)
        in_maps.append(
            {
                "xT": xt,
                "wg": np.ascontiguousarray(wgt[sl]),
                "wu": np.ascontiguousarray(wut[sl]),
                "wd": np.ascontiguousarray(wdt[sl]),
            }
        )
    return in_maps


def kernel(hidden_states, w_gate, w_up, w_down, _trace=False, _trace_kwargs=None):
    from concourse.bass_utils import run_bass_kernel_spmd

    nc = get_program()
    in_maps = _prep_in_maps(hidden_states, w_gate, w_up, w_down)
    kwargs = {}
    if _trace:
        kwargs = dict(trace=True, **(_trace_kwargs or {}))
    res = run_bass_kernel_spmd(nc, in_maps, core_ids=list(range(N_CORES)), **kwargs)
    out = np.concatenate(
        [
            res.results[c]["out"].astype(np.float32).reshape(E_PER * T, H)
            for c in range(N_CORES)
        ],
        axis=0,
    )
    if _trace:
        _CACHE["last_results"] = res
    return out

